# revision 41
# baseline (speedup 1.0000x reference)
"""Trainium2 Bass kernel for nn_Auto_Attn (self-attention + context flow + convs).

Sharding: 8 cores = (batch b in 0..3) x (image half s in 0..1). Each core
computes its half's 32 output rows plus 2 halo rows of the attention output
(conv1 3x3 -> conv2 3x3 needs a 2-row inp halo).

Per-core algorithm (validated against a float64 reference in numpy):
  q = wq @ x (1x1 conv);  energy[m,n] = q_m . q_n  (symmetric since key==query)
  Softmax rows are computed TRANSPOSED (eT[n,m], n on partitions) so the
  attention einsums need no transposes of the big attn matrix:
    - shift-invariance: softmax(e[m,:]) == softmax(e[m,:] - d[m]) for
      d[m] = e[m,m]; Cauchy-Schwarz bounds e[m,n]-d[m] <= (e[n,n]-e[m,m])/2
      so exp never overflows. The shift is folded into the energy matmul
      via an augmented contraction row: q_aug_n = [q; 1], q_aug_m = [q; -d].
    - Z[m] from a ones-column matmul against expT (pz).
  out/cf = expT.T @ [v | pre] in fp8 (e4m3) with DoubleRow perf mode
  (2 contraction rows/cycle); epilogue applies 1/Z, gamma/alpha/mask vectors,
  writes the conv input buffer via PE transposes into a zero-padded
  [38 x 66] bf16 spatial layout.

  Convs are 9-tap shifted bf16 matmuls over the padded layout, emitted in
  row-aligned chunks INTERLEAVED into the attention block loop: conv matmuls
  fill the tensor-engine gaps during the scalar-bound energy->exp phases,
  keeping the PE at full p-state. Outputs stream to DRAM per conv2 chunk.

dtypes: fp32r energy; fp8 e4m3 value matmuls (errors damped by gamma/alpha
~0.1); bf16 everywhere else. Validated offline: rel err ~3e-3 vs 2e-2 gate.
"""
import numpy as np
from contextlib import ExitStack

import ml_dtypes
import concourse.bass as bass
import concourse.tile as tile
from concourse import bacc, mybir
from concourse.bass_utils import run_bass_kernel_spmd
from concourse.masks import make_identity

F32 = mybir.dt.float32
F32R = mybir.dt.float32r
BF16 = mybir.dt.bfloat16
FP8 = mybir.dt.float8e4
DR = mybir.MatmulPerfMode.DoubleRow
EXP = mybir.ActivationFunctionType.Exp
PRELU = mybir.ActivationFunctionType.Prelu
IDENT = mybir.ActivationFunctionType.Identity

B, C, H, W = 4, 256, 64, 64
N = H * W                # 4096
CQ = 64
HALF = 32
WIN = 36                 # window rows (32 + 2 halo each side)
BUFR, BUFC = 38, 66      # padded conv buffer: +1 guard row / +1 pad col each side
MW = WIN * 64            # 2304 m positions per core
NT = N // 128            # 32 n-tiles
MCH = MW // 128          # 18 m-chunks
R1 = [(2, 6), (8, 6), (14, 6), (20, 6), (26, 6), (32, 4)]   # conv1 row chunks
R2 = [(3, 6), (9, 6), (15, 6), (21, 6), (27, 6), (33, 2)]   # conv2 row chunks

_PROGRAM_CACHE = {}


def tap_off(t):
    return (t // 3 - 1) * BUFC + (t % 3 - 1)


# --------------------------------------------------------------------------
# host-side prep
# --------------------------------------------------------------------------

def host_prep(inputs, core):
    b, s = core // 2, core % 2
    f32 = np.float32
    bf16 = ml_dtypes.bfloat16
    fp8 = ml_dtypes.float8_e4m3
    x = np.asarray(inputs['x'][b], f32).reshape(C, N)
    pre = np.asarray(inputs['pre'][b], f32).reshape(C, N)
    M = np.asarray(inputs['mask'][b, 0], f32).reshape(N)
    gamma = float(np.asarray(inputs['gamma']).reshape(-1)[0])
    alpha = float(np.asarray(inputs['alpha']).reshape(-1)[0])

    r_lo = s * HALF - 2
    rows = np.arange(r_lo, r_lo + WIN)
    valid_rows = (rows >= 0) & (rows < H)
    valid = np.repeat(valid_rows, 64).astype(f32)

    def win_slice(t):
        o = np.zeros((t.shape[0], MW), f32)
        vr = np.where(valid_rows)[0]
        o[:, vr[0] * 64:vr[-1] * 64 + 64] = t[:, rows[vr[0]] * 64: rows[vr[-1]] * 64 + 64]
        return o

    xm = win_slice(x)
    pm = win_slice(pre)
    Mw = win_slice(M[None, :])[0]

    vpT = np.zeros((N, 2 * C), f32)
    vpT[:, 0:C] = x.T
    vpT[:, C:2 * C] = pre.T

    xTm = np.ascontiguousarray(xm.T)    # [MW, C]
    pTm = np.ascontiguousarray(pm.T)

    def pervec(v):  # [MW] -> [128, 18] (per-partition layout per m-chunk)
        return np.ascontiguousarray(v.reshape(MCH, 128).T)

    wq = np.asarray(inputs['wq'], f32).reshape(CQ, C)
    w1 = np.asarray(inputs['w1'], f32)
    w2 = np.asarray(inputs['w2'], f32)
    ws = np.asarray(inputs['ws'], f32).reshape(C, 2 * C)
    bq = np.asarray(inputs['bq'], f32).reshape(CQ, 1)
    b1 = np.asarray(inputs['b1'], f32)
    b2 = np.asarray(inputs['b2'], f32)
    bs = np.asarray(inputs['bs'], f32)

    br_rows = r_lo + np.arange(BUFR) - 1
    rv = ((br_rows >= 0) & (br_rows < H)).astype(f32)
    cv = np.zeros(BUFC, f32)
    cv[1:65] = 1.0
    hrmask = np.broadcast_to((rv[:, None] * cv[None, :]).reshape(1, -1),
                             (128, BUFR * BUFC))

    return {
        'xb': x.reshape(2, 128, N).astype(bf16),
        'xm': xm.reshape(2, 128, MW).astype(bf16),
        'vpT': vpT.astype(fp8).reshape(NT, 128, 2 * C),
        'vones': np.ones((128, NT), fp8),
        'xTm': np.ascontiguousarray(xTm.reshape(MCH, 128, C).transpose(1, 0, 2)
                                    .reshape(128, MCH * C)).astype(bf16),
        'pTm': np.ascontiguousarray(pTm.reshape(MCH, 128, C).transpose(1, 0, 2)
                                    .reshape(128, MCH * C)).astype(bf16),
        'avec': pervec(gamma * valid),
        'b1vec': pervec(alpha * Mw * valid),
        'b2vec': pervec((1.0 - Mw) * valid),
        'wq': np.ascontiguousarray(wq.T).reshape(2, 128, CQ).astype(bf16),
        'bq': bq,
        'w1': np.ascontiguousarray(np.transpose(w1, (2, 3, 1, 0))).reshape(9, 4, 128, C).astype(bf16),
        'w2': np.ascontiguousarray(np.transpose(w2, (2, 3, 1, 0))).reshape(9, 2, 128, C).astype(bf16),
        'ws': np.ascontiguousarray(ws.T).reshape(4, 128, C).astype(bf16),
        'b1c': np.ascontiguousarray(b1.reshape(2, 128).T),
        'bfin': np.ascontiguousarray((b2 + bs).reshape(2, 128).T),
        'hrmask': np.ascontiguousarray(hrmask).astype(bf16),
        'ones_row': np.ones((1, N), bf16),
        'ones_col': np.ones((CQ, 1), bf16),
    }


# --------------------------------------------------------------------------
# device program
# --------------------------------------------------------------------------

def build_program():
    nc = bacc.Bacc("TRN2", target_bir_lowering=False, debug=False, num_devices=8)

    def din(name, shape, dt=F32):
        return nc.dram_tensor(name, shape, dt, kind="ExternalInput").ap()

    xb_d = din('xb', [2, 128, N], BF16)
    xm_d = din('xm', [2, 128, MW], BF16)
    vpT_d = din('vpT', [NT, 128, 2 * C], FP8)
    vones_d = din('vones', [128, NT], FP8)
    xTm_d = din('xTm', [128, MCH * C], BF16)
    pTm_d = din('pTm', [128, MCH * C], BF16)
    avec_d = din('avec', [128, MCH])
    b1vec_d = din('b1vec', [128, MCH])
    b2vec_d = din('b2vec', [128, MCH])
    wq_d = din('wq', [2, 128, CQ], BF16)
    bq_d = din('bq', [CQ, 1])
    w1_d = din('w1', [9, 4, 128, C], BF16)
    w2_d = din('w2', [9, 2, 128, C], BF16)
    ws_d = din('ws', [4, 128, C], BF16)
    b1c_d = din('b1c', [128, 2])
    bfin_d = din('bfin', [128, 2])
    hrmask_d = din('hrmask', [128, BUFR * BUFC], BF16)
    onesr_d = din('ones_row', [1, N], BF16)
    onesc_d = din('ones_col', [CQ, 1], BF16)
    y_d = nc.dram_tensor('y', [2, 128, HALF, 64], F32, kind="ExternalOutput").ap()

    with tile.TileContext(nc) as tc, ExitStack() as ctx:
        # ---------------- persistent tiles ----------------
        persist = ctx.enter_context(tc.tile_pool(name="persist", bufs=1))
        inp_raw = [persist.tile([128, BUFR * BUFC], BF16, tag=f"inp{i}", name=f"inp{i}")
                   for i in range(4)]
        h1 = [persist.tile([128, BUFR * BUFC], BF16, tag=f"h1{i}", name=f"h1{i}")
              for i in range(2)]
        w1_t = persist.tile([128, 36 * C], BF16, tag="w1")
        w2_t = persist.tile([128, 18 * C], BF16, tag="w2")
        ws_t = persist.tile([128, 4 * C], BF16, tag="ws")
        b1c_t = persist.tile([128, 2], F32, tag="b1c")
        bfin_t = persist.tile([128, 2], F32, tag="bfin")
        hrm_t = persist.tile([128, BUFR * BUFC], BF16, tag="hrm")
        y_sb = [persist.tile([128, HALF * 64], F32, tag=f"ysb{i}", name=f"ysb{i}")
                for i in range(2)]

        with tc.tile_pool(name="attn", bufs=1) as ap:
            q_aug_n = ap.tile([CQ + 1, N], BF16, tag="qn")
            q_aug_m = ap.tile([CQ + 1, MW], BF16, tag="qm")
            vpT_t = ap.tile([128, NT * 2 * C], FP8, tag="vpT")
            vones_t = ap.tile([128, NT], FP8, tag="vones")
            xTmA = ap.tile([128, MCH * C], BF16, tag="xTmA")
            pTmA = ap.tile([128, MCH * C], BF16, tag="pTmA")
            avec_t = ap.tile([128, MCH], F32, tag="av")
            b1vec_t = ap.tile([128, MCH], F32, tag="b1v")
            b2vec_t = ap.tile([128, MCH], F32, tag="b2v")
            wq_t = ap.tile([128, 2 * CQ], BF16, tag="wq")
            bq_t = ap.tile([CQ, 1], F32, tag="bq")
            onesq = ap.tile([CQ, 1], BF16, tag="onesq")
            ident = ap.tile([128, 128], BF16, tag="id")

            nc.sync.dma_start(bq_t[:], bq_d)
            nc.sync.dma_start(onesq[:], onesc_d)
            for ck in range(2):
                nc.sync.dma_start(wq_t[:, ck * CQ:(ck + 1) * CQ], wq_d[ck])
            nc.sync.dma_start(q_aug_n[CQ:CQ + 1, :], onesr_d)
            make_identity(nc, ident[:])

            # ---------------- q phase ----------------
            with tc.tile_pool(name="qp", bufs=8) as qp, \
                 tc.tile_pool(name="qp1", bufs=1) as qp1, \
                 tc.tile_pool(name="qps", bufs=2, space="PSUM") as qps:
                for nb in range(8):
                    qsum = qps.tile([CQ, 512], F32, tag="qpsum")
                    for ck in range(2):
                        xt = qp.tile([128, 512], BF16, tag=f"x{ck}")
                        nc.sync.dma_start(xt[:], xb_d[ck][:, nb * 512:(nb + 1) * 512])
                        nc.tensor.matmul(qsum[:], wq_t[:, ck * CQ:(ck + 1) * CQ],
                                         xt[:], start=(ck == 0), stop=(ck == 1))
                    nc.vector.tensor_scalar_add(
                        q_aug_n[0:CQ, nb * 512:(nb + 1) * 512], qsum[:], bq_t[:])

                mchunks = [512, 512, 512, 512, 256]
                qsq = qp1.tile([CQ, MW], BF16, tag="qsq")
                mo = 0
                for cs in mchunks:
                    qsum = qps.tile([CQ, 512], F32, tag="qpsum")
                    for ck in range(2):
                        xt = qp.tile([128, 512], BF16, tag=f"x{ck}")
                        nc.sync.dma_start(xt[:, 0:cs], xm_d[ck][:, mo:mo + cs])
                        nc.tensor.matmul(qsum[:, 0:cs], wq_t[:, ck * CQ:(ck + 1) * CQ],
                                         xt[:, 0:cs], start=(ck == 0), stop=(ck == 1))
                    nc.vector.tensor_scalar_add(
                        q_aug_m[0:CQ, mo:mo + cs], qsum[:, 0:cs], bq_t[:])
                    # d for this chunk immediately: overlaps the next xm DMA
                    nc.vector.tensor_mul(qsq[:, mo:mo + cs], q_aug_m[0:CQ, mo:mo + cs],
                                         q_aug_m[0:CQ, mo:mo + cs])
                    dps = qps.tile([1, 512], F32, tag="dpsum")
                    nc.tensor.matmul(dps[:, 0:cs], onesq[:], qsq[:, mo:mo + cs],
                                     start=True, stop=True)
                    nc.vector.tensor_scalar_mul(
                        q_aug_m[CQ:CQ + 1, mo:mo + cs], dps[:, 0:cs], -1.0)
                    mo += cs

                # bulk loads issued now: they queue behind the q streams and
                # overlap the attention phase
                nc.sync.dma_start(vones_t[:], vones_d)
                for nt in range(NT):
                    nc.sync.dma_start(vpT_t[:, nt * 2 * C:(nt + 1) * 2 * C], vpT_d[nt])
                nc.sync.dma_start(xTmA[:], xTm_d)
                nc.sync.dma_start(pTmA[:], pTm_d)
                nc.sync.dma_start(avec_t[:], avec_d)
                nc.sync.dma_start(b1vec_t[:], b1vec_d)
                nc.sync.dma_start(b2vec_t[:], b2vec_d)
                for t in range(9):
                    for ick in range(4):
                        nc.sync.dma_start(w1_t[:, (t * 4 + ick) * C:(t * 4 + ick + 1) * C],
                                          w1_d[t, ick])
                for t in range(9):
                    for ick in range(2):
                        nc.sync.dma_start(w2_t[:, (t * 2 + ick) * C:(t * 2 + ick + 1) * C],
                                          w2_d[t, ick])
                for ick in range(4):
                    nc.sync.dma_start(ws_t[:, ick * C:(ick + 1) * C], ws_d[ick])
                nc.sync.dma_start(hrm_t[:], hrmask_d)
                nc.sync.dma_start(b1c_t[:], b1c_d)
                nc.sync.dma_start(bfin_t[:], bfin_d)

            # padded-buffer zeroing: off the startup critical path, done
            # well before the first epilogue/conv reads
            for t_ in inp_raw + h1:
                nc.vector.memset(t_[:], 0.0)

            # ---------------- attention blocks + interleaved convs ---------
            with tc.tile_pool(name="expp", bufs=2) as expp, \
                 tc.tile_pool(name="ep", bufs=2) as epp, \
                 tc.tile_pool(name="lrp", bufs=2) as lrp, \
                 tc.tile_pool(name="eps", bufs=2, space="PSUM") as eps, \
                 tc.tile_pool(name="ops", bufs=2, space="PSUM") as ops, \
                 tc.tile_pool(name="zps", bufs=1, space="PSUM") as zps, \
                 tc.tile_pool(name="tps", bufs=1, space="PSUM") as tps, \
                 tc.tile_pool(name="c1ps", bufs=1, space="PSUM") as c1ps, \
                 tc.tile_pool(name="c2ps", bufs=1, space="PSUM") as c2ps:

                def conv1_chunk(k):
                    r0, nr = R1[k]
                    base, cs = r0 * BUFC, nr * BUFC
                    lr = [lrp.tile([128, 536], BF16, tag=f"lr{i}", name=f"lr{i}")
                          for i in range(4)]
                    lrt = lrp.tile([128, 536], BF16, tag="lrt", name="lrt")
                    for ick in range(2):
                        yield nc.scalar.activation(lr[ick][:, 0:cs + 134],
                                                   inp_raw[ick][:, base - 67:base + cs + 67],
                                                   PRELU, alpha=0.1)
                    for ick in range(2, 4):
                        # prelu on vector: max(x,0) + 0.1*min(x,0)
                        src = inp_raw[ick][:, base - 67:base + cs + 67]
                        yield nc.vector.tensor_scalar_max(lr[ick][:, 0:cs + 134], src, 0.0)
                        yield nc.vector.tensor_scalar_min(lrt[:, 0:cs + 134], src, 0.0)
                        yield nc.vector.scalar_tensor_tensor(
                            lr[ick][:, 0:cs + 134], lrt[:, 0:cs + 134], 0.1,
                            lr[ick][:, 0:cs + 134],
                            op0=mybir.AluOpType.mult, op1=mybir.AluOpType.add)
                    for oc in range(2):
                        ps1 = c1ps.tile([128, 396], F32, tag="ps1")
                        k_ = 0
                        for t in range(9):
                            off = 67 + tap_off(t)
                            for ick in range(4):
                                yield nc.tensor.matmul(
                                    ps1[:, 0:cs],
                                    w1_t[:, (t * 4 + ick) * C + oc * 128:
                                         (t * 4 + ick) * C + oc * 128 + 128],
                                    lr[ick][:, off:off + cs],
                                    start=(k_ == 0), stop=(k_ == 35))
                                k_ += 1
                        yield nc.scalar.activation(h1[oc][:, base:base + cs], ps1[:, 0:cs],
                                                   PRELU, bias=b1c_t[:, oc:oc + 1],
                                                   alpha=0.1)
                        yield nc.vector.tensor_mul(h1[oc][:, base:base + cs],
                                                   h1[oc][:, base:base + cs],
                                                   hrm_t[:, base:base + cs])

                def conv2_chunk(k):
                    r0, nr = R2[k]
                    base, cs = r0 * BUFC, nr * BUFC
                    for oc in range(2):
                        ps2 = c2ps.tile([128, 396], F32, tag="ps2")
                        k_ = 0
                        for ick in range(4):
                            yield nc.tensor.matmul(
                                ps2[:, 0:cs],
                                ws_t[:, ick * C + oc * 128:ick * C + oc * 128 + 128],
                                inp_raw[ick][:, base:base + cs],
                                start=(k_ == 0), stop=False)
                            k_ += 1
                        for t in range(9):
                            off = tap_off(t)
                            for ick in range(2):
                                k_ += 1
                                yield nc.tensor.matmul(
                                    ps2[:, 0:cs],
                                    w2_t[:, (t * 2 + ick) * C + oc * 128:
                                         (t * 2 + ick) * C + oc * 128 + 128],
                                    h1[ick][:, base + off:base + off + cs],
                                    start=False, stop=(k_ == 22))
                        src = ps2[:, 0:cs].rearrange("p (r c) -> p r c", c=BUFC)[:, :, 1:65]
                        dst = y_sb[oc][:, (r0 - 3) * 64:(r0 - 3 + nr) * 64] \
                            .rearrange("p (r c) -> p r c", c=64)
                        yield nc.vector.tensor_scalar_add(dst, src,
                                                          bfin_t[:, oc:oc + 1])
                        yield nc.sync.dma_start(
                            y_d[oc][:, r0 - 3:r0 - 3 + nr, :], dst)

                conv_queue = []

                def pull_conv(n):
                    done = 0
                    while conv_queue and done < n:
                        try:
                            next(conv_queue[0])
                            done += 1
                        except StopIteration:
                            conv_queue.pop(0)

                ready = {0: [(conv1_chunk, 0)],
                         1: [(conv1_chunk, 1), (conv2_chunk, 0)],
                         2: [(conv1_chunk, 2), (conv2_chunk, 1)],
                         3: [(conv1_chunk, 3), (conv2_chunk, 2),
                             (conv1_chunk, 4), (conv2_chunk, 3)],
                         4: [(conv1_chunk, 5), (conv2_chunk, 4), (conv2_chunk, 5)]}

                blocks = [(0, 512), (512, 512), (1024, 512), (1536, 512), (2048, 256)]
                vp3 = vpT_t[:].rearrange("p (t v) -> p t v", v=2 * C)
                def value_chain(exp3, bs, m0, mc_lo=0, mc_hi=None):
                    for mc in range(mc_lo, mc_hi if mc_hi is not None else bs // 128):
                        j = m0 // 128 + mc
                        po = ops.tile([128, 512], F32, tag="po")
                        pz = zps.tile([128, 1], F32, tag="pz")
                        for t in range(NT // 2):
                            lhs = exp3[:, 2 * t:2 * t + 2, mc * 128:mc * 128 + 128]
                            nc.tensor.matmul(
                                po[:], lhs, vp3[:, 2 * t:2 * t + 2, :],
                                start=(t == 0), stop=(t == NT // 2 - 1),
                                perf_mode=DR)
                            for u in (2 * t, 2 * t + 1):
                                nc.tensor.matmul(
                                    pz[:], exp3[:, u, mc * 128:mc * 128 + 128],
                                    vones_t[:, u:u + 1],
                                    start=(u == 0), stop=(u == NT - 1))
                            yield
                        # epilogue
                        rt = epp.tile([128, 1], F32, tag="rt")
                        nc.vector.reciprocal(rt[:], pz[:, 0:1])
                        s1 = epp.tile([128, 1], F32, tag="s1")
                        s2 = epp.tile([128, 1], F32, tag="s2")
                        nc.vector.tensor_mul(s1[:], rt[:], avec_t[:, j:j + 1])
                        nc.vector.tensor_mul(s2[:], rt[:], b1vec_t[:, j:j + 1])
                        yield
                        ov = epp.tile([128, C], BF16, tag="ov")
                        cv = epp.tile([128, C], BF16, tag="cv")
                        tv = epp.tile([128, C], BF16, tag="tv")
                        nc.vector.tensor_scalar_mul(ov[:], po[:, 0:C], s1[:])
                        nc.vector.tensor_add(ov[:], ov[:], xTmA[:, j * C:(j + 1) * C])
                        yield
                        nc.vector.tensor_scalar_mul(cv[:], po[:, C:2 * C], s2[:])
                        nc.vector.tensor_scalar_mul(tv[:], pTmA[:, j * C:(j + 1) * C],
                                                    b2vec_t[:, j:j + 1])
                        nc.vector.tensor_add(cv[:], cv[:], tv[:])
                        yield
                        # transpose [m, c] -> [c, m] into padded conv layout
                        br0 = 1 + 2 * j
                        for h_ in range(4):
                            src_ = (ov if h_ < 2 else cv)
                            pt = tps.tile([128, 128], BF16, tag="pt")
                            nc.tensor.transpose(pt[:], src_[:, (h_ % 2) * 128:(h_ % 2) * 128 + 128],
                                                ident[:])
                            dst = inp_raw[h_][:].rearrange("p (r c) -> p r c", c=BUFC)
                            nc.vector.tensor_copy(dst[:, br0:br0 + 2, 1:65],
                                                  pt[:].rearrange("p (r c) -> p r c", c=64))
                            yield

                # unified work queue: value/epilogue generators go to the
                # front (they unblock conv-input rows), conv chunks append
                def pull_work(n):
                    done = 0
                    while conv_queue and done < n:
                        try:
                            next(conv_queue[0])
                            done += 1
                        except StopIteration:
                            conv_queue.pop(0)

                val_q = []

                def pull_any(n):
                    done = 0
                    while done < n and (val_q or conv_queue):
                        q_ = val_q if val_q else conv_queue
                        try:
                            next(q_[0])
                            done += 1
                        except StopIteration:
                            q_.pop(0)

                prev_vals = None
                for bi, (m0, bs) in enumerate(blocks):
                    expT = expp.tile([128, NT * 512], FP8, tag="expT", name="expT")
                    exp3 = expT[:].rearrange("p (t b) -> p t b", b=bs)
                    if prev_vals is not None:
                        val_q.append(prev_vals)
                    vc0 = value_chain(exp3, bs, m0, 0, 1) if bi == 0 else None
                    for nt in range(NT):
                        ets = eps.tile([128, bs], F32, tag="et", name="ets")
                        nc.tensor.matmul(ets[:], q_aug_n[:, nt * 128:(nt + 1) * 128],
                                         q_aug_m[:, m0:m0 + bs],
                                         start=True, stop=True)
                        nc.scalar.activation(expT[:, nt * bs:(nt + 1) * bs], ets[:], EXP)
                        # fill exp-paced tensor gaps with the previous
                        # block's value chains + pending conv matmuls
                        if vc0 is not None and nt % 2 == 1 and nt >= 3:
                            next(vc0, None)
                        pull_any(4)
                    # only one value generator may be live (single pz bank):
                    # finish this block's leftovers before queueing the next
                    if vc0 is not None:
                        for _ in vc0:
                            pass
                    while val_q:
                        pull_any(1)
                    prev_vals = value_chain(exp3, bs, m0, 1 if bi == 0 else 0)
                    for fn, k in ready.get(bi, []):
                        conv_queue.append(fn(k))
                # drain the last block's values and remaining conv work
                val_q.append(prev_vals)
                while val_q:
                    pull_any(1)
                pull_any(10 ** 9)

    nc.compile()
    return nc

# --------------------------------------------------------------------------
# entry point
# --------------------------------------------------------------------------

def _get_program():
    if 'nc' not in _PROGRAM_CACHE:
        _PROGRAM_CACHE['nc'] = build_program()
    return _PROGRAM_CACHE['nc']


def kernel(_trace=False, **inputs):
    nc = _get_program()
    in_maps = [host_prep(inputs, core) for core in range(8)]
    res = run_bass_kernel_spmd(nc, in_maps, core_ids=list(range(8)),
                               trace=_trace)
    y = np.zeros((B, C, H, W), np.float32)
    for core in range(8):
        b, s = core // 2, core % 2
        yh = res.results[core]['y']        # [2, 128, 32, 64]
        y[b, :, s * HALF:(s + 1) * HALF, :] = yh.reshape(C, HALF, 64)
    if _trace:
        return y, res
    return y


# revision 42
# speedup vs baseline: 1.0150x; 1.0150x over previous
"""Trainium2 Bass kernel for nn_Auto_Attn (self-attention + context flow + convs).

Sharding: 8 cores = (batch b in 0..3) x (image half s in 0..1). Each core
computes its half's 32 output rows plus 2 halo rows of the attention output
(conv1 3x3 -> conv2 3x3 needs a 2-row inp halo).

Per-core algorithm (validated against a float64 reference in numpy):
  q = wq @ x (1x1 conv);  energy[m,n] = q_m . q_n  (symmetric since key==query)
  Softmax rows are computed TRANSPOSED (eT[n,m], n on partitions) so the
  attention einsums need no transposes of the big attn matrix:
    - shift-invariance: softmax(e[m,:]) == softmax(e[m,:] - d[m]) for
      d[m] = e[m,m]; Cauchy-Schwarz bounds e[m,n]-d[m] <= (e[n,n]-e[m,m])/2
      so exp never overflows. The shift is folded into the energy matmul
      via an augmented contraction row: q_aug_n = [q; 1], q_aug_m = [q; -d].
    - Z[m] from a ones-column matmul against expT (pz).
  out/cf = expT.T @ [v | pre] in fp8 (e4m3) with DoubleRow perf mode
  (2 contraction rows/cycle); epilogue applies 1/Z, gamma/alpha/mask vectors,
  writes the conv input buffer via PE transposes into a zero-padded
  [38 x 66] bf16 spatial layout.

  Convs are 9-tap shifted bf16 matmuls over the padded layout, emitted in
  row-aligned chunks INTERLEAVED into the attention block loop: conv matmuls
  fill the tensor-engine gaps during the scalar-bound energy->exp phases,
  keeping the PE at full p-state. Outputs stream to DRAM per conv2 chunk.

dtypes: fp32r energy; fp8 e4m3 value matmuls (errors damped by gamma/alpha
~0.1); bf16 everywhere else. Validated offline: rel err ~3e-3 vs 2e-2 gate.
"""
import numpy as np
from contextlib import ExitStack

import ml_dtypes
import concourse.bass as bass
import concourse.tile as tile
from concourse import bacc, mybir
from concourse.bass_utils import run_bass_kernel_spmd
from concourse.masks import make_identity

F32 = mybir.dt.float32
F32R = mybir.dt.float32r
BF16 = mybir.dt.bfloat16
FP8 = mybir.dt.float8e4
DR = mybir.MatmulPerfMode.DoubleRow
EXP = mybir.ActivationFunctionType.Exp
PRELU = mybir.ActivationFunctionType.Prelu
IDENT = mybir.ActivationFunctionType.Identity

B, C, H, W = 4, 256, 64, 64
N = H * W                # 4096
CQ = 64
HALF = 32
WIN = 36                 # window rows (32 + 2 halo each side)
BUFR, BUFC = 38, 66      # padded conv buffer: +1 guard row / +1 pad col each side
MW = WIN * 64            # 2304 m positions per core
NT = N // 128            # 32 n-tiles
MCH = MW // 128          # 18 m-chunks
R1 = [(2, 6), (8, 6), (14, 6), (20, 6), (26, 6), (32, 4)]   # conv1 row chunks
R2 = [(3, 6), (9, 6), (15, 6), (21, 6), (27, 6), (33, 2)]   # conv2 row chunks

_PROGRAM_CACHE = {}


def tap_off(t):
    return (t // 3 - 1) * BUFC + (t % 3 - 1)


# --------------------------------------------------------------------------
# host-side prep
# --------------------------------------------------------------------------

def host_prep(inputs, core):
    b, s = core // 2, core % 2
    f32 = np.float32
    bf16 = ml_dtypes.bfloat16
    fp8 = ml_dtypes.float8_e4m3
    x = np.asarray(inputs['x'][b], f32).reshape(C, N)
    pre = np.asarray(inputs['pre'][b], f32).reshape(C, N)
    M = np.asarray(inputs['mask'][b, 0], f32).reshape(N)
    gamma = float(np.asarray(inputs['gamma']).reshape(-1)[0])
    alpha = float(np.asarray(inputs['alpha']).reshape(-1)[0])

    r_lo = s * HALF - 2
    rows = np.arange(r_lo, r_lo + WIN)
    valid_rows = (rows >= 0) & (rows < H)
    valid = np.repeat(valid_rows, 64).astype(f32)

    def win_slice(t):
        o = np.zeros((t.shape[0], MW), f32)
        vr = np.where(valid_rows)[0]
        o[:, vr[0] * 64:vr[-1] * 64 + 64] = t[:, rows[vr[0]] * 64: rows[vr[-1]] * 64 + 64]
        return o

    xm = win_slice(x)
    pm = win_slice(pre)
    Mw = win_slice(M[None, :])[0]

    vpT = np.zeros((N, 2 * C), f32)
    vpT[:, 0:C] = x.T
    vpT[:, C:2 * C] = pre.T

    xTm = np.ascontiguousarray(xm.T)    # [MW, C]
    pTm = np.ascontiguousarray(pm.T)

    def pervec(v):  # [MW] -> [128, 18] (per-partition layout per m-chunk)
        return np.ascontiguousarray(v.reshape(MCH, 128).T)

    wq = np.asarray(inputs['wq'], f32).reshape(CQ, C)
    w1 = np.asarray(inputs['w1'], f32)
    w2 = np.asarray(inputs['w2'], f32)
    ws = np.asarray(inputs['ws'], f32).reshape(C, 2 * C)
    bq = np.asarray(inputs['bq'], f32).reshape(CQ, 1)
    b1 = np.asarray(inputs['b1'], f32)
    b2 = np.asarray(inputs['b2'], f32)
    bs = np.asarray(inputs['bs'], f32)

    br_rows = r_lo + np.arange(BUFR) - 1
    rv = ((br_rows >= 0) & (br_rows < H)).astype(f32)
    cv = np.zeros(BUFC, f32)
    cv[1:65] = 1.0
    hrmask = np.broadcast_to((rv[:, None] * cv[None, :]).reshape(1, -1),
                             (128, BUFR * BUFC))

    return {
        'xb': x.reshape(2, 128, N).astype(bf16),
        'xm': xm.reshape(2, 128, MW).astype(bf16),
        'vpT': vpT.astype(fp8).reshape(NT, 128, 2 * C),
        'vones': np.ones((128, NT), fp8),
        'xTm': np.ascontiguousarray(xTm.reshape(MCH, 128, C).transpose(1, 0, 2)
                                    .reshape(128, MCH * C)).astype(bf16),
        'pTm': np.ascontiguousarray(pTm.reshape(MCH, 128, C).transpose(1, 0, 2)
                                    .reshape(128, MCH * C)).astype(bf16),
        'avec': pervec(gamma * valid),
        'b1vec': pervec(alpha * Mw * valid),
        'b2vec': pervec((1.0 - Mw) * valid),
        'wq': np.ascontiguousarray(wq.T).reshape(2, 128, CQ).astype(bf16),
        'bq': bq,
        'w1': np.ascontiguousarray(np.transpose(w1, (2, 3, 1, 0))).reshape(9, 4, 128, C).astype(bf16),
        'w2': np.ascontiguousarray(np.transpose(w2, (2, 3, 1, 0))).reshape(9, 2, 128, C).astype(bf16),
        'ws': np.ascontiguousarray(ws.T).reshape(4, 128, C).astype(bf16),
        'b1c': np.ascontiguousarray(b1.reshape(2, 128).T),
        'bfin': np.ascontiguousarray((b2 + bs).reshape(2, 128).T),
        'hrmask': np.ascontiguousarray(hrmask).astype(bf16),
        'ones_row': np.ones((1, N), bf16),
        'ones_col': np.ones((CQ, 1), bf16),
    }


# --------------------------------------------------------------------------
# device program
# --------------------------------------------------------------------------

def build_program():
    nc = bacc.Bacc("TRN2", target_bir_lowering=False, debug=False, num_devices=8)

    def din(name, shape, dt=F32):
        return nc.dram_tensor(name, shape, dt, kind="ExternalInput").ap()

    xb_d = din('xb', [2, 128, N], BF16)
    xm_d = din('xm', [2, 128, MW], BF16)
    vpT_d = din('vpT', [NT, 128, 2 * C], FP8)
    vones_d = din('vones', [128, NT], FP8)
    xTm_d = din('xTm', [128, MCH * C], BF16)
    pTm_d = din('pTm', [128, MCH * C], BF16)
    avec_d = din('avec', [128, MCH])
    b1vec_d = din('b1vec', [128, MCH])
    b2vec_d = din('b2vec', [128, MCH])
    wq_d = din('wq', [2, 128, CQ], BF16)
    bq_d = din('bq', [CQ, 1])
    w1_d = din('w1', [9, 4, 128, C], BF16)
    w2_d = din('w2', [9, 2, 128, C], BF16)
    ws_d = din('ws', [4, 128, C], BF16)
    b1c_d = din('b1c', [128, 2])
    bfin_d = din('bfin', [128, 2])
    hrmask_d = din('hrmask', [128, BUFR * BUFC], BF16)
    onesr_d = din('ones_row', [1, N], BF16)
    onesc_d = din('ones_col', [CQ, 1], BF16)
    y_d = nc.dram_tensor('y', [2, 128, HALF, 64], F32, kind="ExternalOutput").ap()

    with tile.TileContext(nc) as tc, ExitStack() as ctx:
        # ---------------- persistent tiles ----------------
        persist = ctx.enter_context(tc.tile_pool(name="persist", bufs=1))
        inp_raw = [persist.tile([128, BUFR * BUFC], BF16, tag=f"inp{i}", name=f"inp{i}")
                   for i in range(4)]
        h1 = [persist.tile([128, BUFR * BUFC], BF16, tag=f"h1{i}", name=f"h1{i}")
              for i in range(2)]
        w1_t = persist.tile([128, 36 * C], BF16, tag="w1")
        w2_t = persist.tile([128, 18 * C], BF16, tag="w2")
        ws_t = persist.tile([128, 4 * C], BF16, tag="ws")
        b1c_t = persist.tile([128, 2], F32, tag="b1c")
        bfin_t = persist.tile([128, 2], F32, tag="bfin")
        hrm_t = persist.tile([128, BUFR * BUFC], BF16, tag="hrm")
        y_sb = [persist.tile([128, HALF * 64], F32, tag=f"ysb{i}", name=f"ysb{i}")
                for i in range(2)]

        with tc.tile_pool(name="attn", bufs=1) as ap:
            q_aug_n = ap.tile([CQ + 1, N], BF16, tag="qn")
            q_aug_m = ap.tile([CQ + 1, MW], BF16, tag="qm")
            vpT_t = ap.tile([128, NT * 2 * C], FP8, tag="vpT")
            vones_t = ap.tile([128, NT], FP8, tag="vones")
            xTmA = ap.tile([128, MCH * C], BF16, tag="xTmA")
            pTmA = ap.tile([128, MCH * C], BF16, tag="pTmA")
            avec_t = ap.tile([128, MCH], F32, tag="av")
            b1vec_t = ap.tile([128, MCH], F32, tag="b1v")
            b2vec_t = ap.tile([128, MCH], F32, tag="b2v")
            wq_t = ap.tile([128, 2 * CQ], BF16, tag="wq")
            xb_t = ap.tile([128, 2 * N], BF16, tag="xbt")
            xm_t = ap.tile([128, 2 * MW], BF16, tag="xmt")
            bq_t = ap.tile([CQ, 1], F32, tag="bq")
            onesq = ap.tile([CQ, 1], BF16, tag="onesq")
            ident = ap.tile([128, 128], BF16, tag="id")

            nc.sync.dma_start(bq_t[:], bq_d)
            nc.sync.dma_start(onesq[:], onesc_d)
            for ck in range(2):
                nc.sync.dma_start(wq_t[:, ck * CQ:(ck + 1) * CQ], wq_d[ck])
            nc.sync.dma_start(q_aug_n[CQ:CQ + 1, :], onesr_d)
            make_identity(nc, ident[:])

            # ---------------- q phase ----------------
            with tc.tile_pool(name="qp1", bufs=1) as qp1, \
                 tc.tile_pool(name="qps", bufs=2, space="PSUM") as qps:
                # coarse input streams: 256KB+ transfers, interleaved so the
                # first q matmuls unblock as early as possible
                QCH = 1024
                for i in range(N // QCH):
                    for ck in range(2):
                        nc.sync.dma_start(
                            xb_t[:, ck * N + i * QCH:ck * N + (i + 1) * QCH],
                            xb_d[ck][:, i * QCH:(i + 1) * QCH])
                for i in range(2):
                    for ck in range(2):
                        nc.sync.dma_start(
                            xm_t[:, ck * MW + i * (MW // 2):ck * MW + (i + 1) * (MW // 2)],
                            xm_d[ck][:, i * (MW // 2):(i + 1) * (MW // 2)])
                for nb in range(8):
                    qsum = qps.tile([CQ, 512], F32, tag="qpsum")
                    for ck in range(2):
                        nc.tensor.matmul(qsum[:], wq_t[:, ck * CQ:(ck + 1) * CQ],
                                         xb_t[:, ck * N + nb * 512:ck * N + (nb + 1) * 512],
                                         start=(ck == 0), stop=(ck == 1))
                    nc.vector.tensor_scalar_add(
                        q_aug_n[0:CQ, nb * 512:(nb + 1) * 512], qsum[:], bq_t[:])

                mchunks = [512, 512, 512, 512, 256]
                qsq = qp1.tile([CQ, MW], BF16, tag="qsq")
                mo = 0
                for cs in mchunks:
                    qsum = qps.tile([CQ, 512], F32, tag="qpsum")
                    for ck in range(2):
                        nc.tensor.matmul(qsum[:, 0:cs], wq_t[:, ck * CQ:(ck + 1) * CQ],
                                         xm_t[:, ck * MW + mo:ck * MW + mo + cs],
                                         start=(ck == 0), stop=(ck == 1))
                    nc.vector.tensor_scalar_add(
                        q_aug_m[0:CQ, mo:mo + cs], qsum[:, 0:cs], bq_t[:])
                    # d for this chunk immediately: overlaps the next xm DMA
                    nc.vector.tensor_mul(qsq[:, mo:mo + cs], q_aug_m[0:CQ, mo:mo + cs],
                                         q_aug_m[0:CQ, mo:mo + cs])
                    dps = qps.tile([1, 512], F32, tag="dpsum")
                    nc.tensor.matmul(dps[:, 0:cs], onesq[:], qsq[:, mo:mo + cs],
                                     start=True, stop=True)
                    nc.vector.tensor_scalar_mul(
                        q_aug_m[CQ:CQ + 1, mo:mo + cs], dps[:, 0:cs], -1.0)
                    mo += cs

                # bulk loads issued now: they queue behind the q streams and
                # overlap the attention phase
                nc.sync.dma_start(vones_t[:], vones_d)
                for nt in range(NT):
                    nc.sync.dma_start(vpT_t[:, nt * 2 * C:(nt + 1) * 2 * C], vpT_d[nt])
                nc.sync.dma_start(xTmA[:], xTm_d)
                nc.sync.dma_start(pTmA[:], pTm_d)
                nc.sync.dma_start(avec_t[:], avec_d)
                nc.sync.dma_start(b1vec_t[:], b1vec_d)
                nc.sync.dma_start(b2vec_t[:], b2vec_d)
                for t in range(9):
                    for ick in range(4):
                        nc.sync.dma_start(w1_t[:, (t * 4 + ick) * C:(t * 4 + ick + 1) * C],
                                          w1_d[t, ick])
                for t in range(9):
                    for ick in range(2):
                        nc.sync.dma_start(w2_t[:, (t * 2 + ick) * C:(t * 2 + ick + 1) * C],
                                          w2_d[t, ick])
                for ick in range(4):
                    nc.sync.dma_start(ws_t[:, ick * C:(ick + 1) * C], ws_d[ick])
                nc.sync.dma_start(hrm_t[:], hrmask_d)
                nc.sync.dma_start(b1c_t[:], b1c_d)
                nc.sync.dma_start(bfin_t[:], bfin_d)

            # padded-buffer zeroing: off the startup critical path, done
            # well before the first epilogue/conv reads
            for t_ in inp_raw + h1:
                nc.vector.memset(t_[:], 0.0)

            # ---------------- attention blocks + interleaved convs ---------
            with tc.tile_pool(name="expp", bufs=2) as expp, \
                 tc.tile_pool(name="ep", bufs=2) as epp, \
                 tc.tile_pool(name="lrp", bufs=2) as lrp, \
                 tc.tile_pool(name="eps", bufs=2, space="PSUM") as eps, \
                 tc.tile_pool(name="ops", bufs=2, space="PSUM") as ops, \
                 tc.tile_pool(name="zps", bufs=1, space="PSUM") as zps, \
                 tc.tile_pool(name="tps", bufs=1, space="PSUM") as tps, \
                 tc.tile_pool(name="c1ps", bufs=1, space="PSUM") as c1ps, \
                 tc.tile_pool(name="c2ps", bufs=1, space="PSUM") as c2ps:

                def conv1_chunk(k):
                    r0, nr = R1[k]
                    base, cs = r0 * BUFC, nr * BUFC
                    lr = [lrp.tile([128, 536], BF16, tag=f"lr{i}", name=f"lr{i}")
                          for i in range(4)]
                    lrt = lrp.tile([128, 536], BF16, tag="lrt", name="lrt")
                    for ick in range(2):
                        yield nc.scalar.activation(lr[ick][:, 0:cs + 134],
                                                   inp_raw[ick][:, base - 67:base + cs + 67],
                                                   PRELU, alpha=0.1)
                    for ick in range(2, 4):
                        # prelu on vector: max(x,0) + 0.1*min(x,0)
                        src = inp_raw[ick][:, base - 67:base + cs + 67]
                        yield nc.vector.tensor_scalar_max(lr[ick][:, 0:cs + 134], src, 0.0)
                        yield nc.vector.tensor_scalar_min(lrt[:, 0:cs + 134], src, 0.0)
                        yield nc.vector.scalar_tensor_tensor(
                            lr[ick][:, 0:cs + 134], lrt[:, 0:cs + 134], 0.1,
                            lr[ick][:, 0:cs + 134],
                            op0=mybir.AluOpType.mult, op1=mybir.AluOpType.add)
                    for oc in range(2):
                        ps1 = c1ps.tile([128, 396], F32, tag="ps1")
                        k_ = 0
                        for t in range(9):
                            off = 67 + tap_off(t)
                            for ick in range(4):
                                yield nc.tensor.matmul(
                                    ps1[:, 0:cs],
                                    w1_t[:, (t * 4 + ick) * C + oc * 128:
                                         (t * 4 + ick) * C + oc * 128 + 128],
                                    lr[ick][:, off:off + cs],
                                    start=(k_ == 0), stop=(k_ == 35))
                                k_ += 1
                        yield nc.scalar.activation(h1[oc][:, base:base + cs], ps1[:, 0:cs],
                                                   PRELU, bias=b1c_t[:, oc:oc + 1],
                                                   alpha=0.1)
                        yield nc.vector.tensor_mul(h1[oc][:, base:base + cs],
                                                   h1[oc][:, base:base + cs],
                                                   hrm_t[:, base:base + cs])

                def conv2_chunk(k):
                    r0, nr = R2[k]
                    base, cs = r0 * BUFC, nr * BUFC
                    for oc in range(2):
                        ps2 = c2ps.tile([128, 396], F32, tag="ps2")
                        k_ = 0
                        for ick in range(4):
                            yield nc.tensor.matmul(
                                ps2[:, 0:cs],
                                ws_t[:, ick * C + oc * 128:ick * C + oc * 128 + 128],
                                inp_raw[ick][:, base:base + cs],
                                start=(k_ == 0), stop=False)
                            k_ += 1
                        for t in range(9):
                            off = tap_off(t)
                            for ick in range(2):
                                k_ += 1
                                yield nc.tensor.matmul(
                                    ps2[:, 0:cs],
                                    w2_t[:, (t * 2 + ick) * C + oc * 128:
                                         (t * 2 + ick) * C + oc * 128 + 128],
                                    h1[ick][:, base + off:base + off + cs],
                                    start=False, stop=(k_ == 22))
                        src = ps2[:, 0:cs].rearrange("p (r c) -> p r c", c=BUFC)[:, :, 1:65]
                        dst = y_sb[oc][:, (r0 - 3) * 64:(r0 - 3 + nr) * 64] \
                            .rearrange("p (r c) -> p r c", c=64)
                        yield nc.vector.tensor_scalar_add(dst, src,
                                                          bfin_t[:, oc:oc + 1])
                        yield nc.sync.dma_start(
                            y_d[oc][:, r0 - 3:r0 - 3 + nr, :], dst)

                conv_queue = []

                def pull_conv(n):
                    done = 0
                    while conv_queue and done < n:
                        try:
                            next(conv_queue[0])
                            done += 1
                        except StopIteration:
                            conv_queue.pop(0)

                ready = {0: [(conv1_chunk, 0)],
                         1: [(conv1_chunk, 1), (conv2_chunk, 0)],
                         2: [(conv1_chunk, 2), (conv2_chunk, 1)],
                         3: [(conv1_chunk, 3), (conv2_chunk, 2),
                             (conv1_chunk, 4), (conv2_chunk, 3)],
                         4: [(conv1_chunk, 5), (conv2_chunk, 4), (conv2_chunk, 5)]}

                blocks = [(0, 512), (512, 512), (1024, 512), (1536, 512), (2048, 256)]
                vp3 = vpT_t[:].rearrange("p (t v) -> p t v", v=2 * C)
                def value_chain(exp3, bs, m0, mc_lo=0, mc_hi=None):
                    for mc in range(mc_lo, mc_hi if mc_hi is not None else bs // 128):
                        j = m0 // 128 + mc
                        po = ops.tile([128, 512], F32, tag="po")
                        pz = zps.tile([128, 1], F32, tag="pz")
                        for t in range(NT // 2):
                            lhs = exp3[:, 2 * t:2 * t + 2, mc * 128:mc * 128 + 128]
                            nc.tensor.matmul(
                                po[:], lhs, vp3[:, 2 * t:2 * t + 2, :],
                                start=(t == 0), stop=(t == NT // 2 - 1),
                                perf_mode=DR)
                            for u in (2 * t, 2 * t + 1):
                                nc.tensor.matmul(
                                    pz[:], exp3[:, u, mc * 128:mc * 128 + 128],
                                    vones_t[:, u:u + 1],
                                    start=(u == 0), stop=(u == NT - 1))
                            yield
                        # epilogue
                        rt = epp.tile([128, 1], F32, tag="rt")
                        nc.vector.reciprocal(rt[:], pz[:, 0:1])
                        s1 = epp.tile([128, 1], F32, tag="s1")
                        s2 = epp.tile([128, 1], F32, tag="s2")
                        nc.vector.tensor_mul(s1[:], rt[:], avec_t[:, j:j + 1])
                        nc.vector.tensor_mul(s2[:], rt[:], b1vec_t[:, j:j + 1])
                        yield
                        ov = epp.tile([128, C], BF16, tag="ov")
                        cv = epp.tile([128, C], BF16, tag="cv")
                        tv = epp.tile([128, C], BF16, tag="tv")
                        nc.vector.tensor_scalar_mul(ov[:], po[:, 0:C], s1[:])
                        nc.vector.tensor_add(ov[:], ov[:], xTmA[:, j * C:(j + 1) * C])
                        yield
                        nc.vector.tensor_scalar_mul(cv[:], po[:, C:2 * C], s2[:])
                        nc.vector.tensor_scalar_mul(tv[:], pTmA[:, j * C:(j + 1) * C],
                                                    b2vec_t[:, j:j + 1])
                        nc.vector.tensor_add(cv[:], cv[:], tv[:])
                        yield
                        # transpose [m, c] -> [c, m] into padded conv layout
                        br0 = 1 + 2 * j
                        for h_ in range(4):
                            src_ = (ov if h_ < 2 else cv)
                            pt = tps.tile([128, 128], BF16, tag="pt")
                            nc.tensor.transpose(pt[:], src_[:, (h_ % 2) * 128:(h_ % 2) * 128 + 128],
                                                ident[:])
                            dst = inp_raw[h_][:].rearrange("p (r c) -> p r c", c=BUFC)
                            nc.vector.tensor_copy(dst[:, br0:br0 + 2, 1:65],
                                                  pt[:].rearrange("p (r c) -> p r c", c=64))
                            yield

                # unified work queue: value/epilogue generators go to the
                # front (they unblock conv-input rows), conv chunks append
                def pull_work(n):
                    done = 0
                    while conv_queue and done < n:
                        try:
                            next(conv_queue[0])
                            done += 1
                        except StopIteration:
                            conv_queue.pop(0)

                val_q = []

                def pull_any(n):
                    done = 0
                    while done < n and (val_q or conv_queue):
                        q_ = val_q if val_q else conv_queue
                        try:
                            next(q_[0])
                            done += 1
                        except StopIteration:
                            q_.pop(0)

                prev_vals = None
                for bi, (m0, bs) in enumerate(blocks):
                    expT = expp.tile([128, NT * 512], FP8, tag="expT", name="expT")
                    exp3 = expT[:].rearrange("p (t b) -> p t b", b=bs)
                    if prev_vals is not None:
                        val_q.append(prev_vals)
                    vc0 = value_chain(exp3, bs, m0, 0, 1) if bi == 0 else None
                    for nt in range(NT):
                        ets = eps.tile([128, bs], F32, tag="et", name="ets")
                        nc.tensor.matmul(ets[:], q_aug_n[:, nt * 128:(nt + 1) * 128],
                                         q_aug_m[:, m0:m0 + bs],
                                         start=True, stop=True)
                        nc.scalar.activation(expT[:, nt * bs:(nt + 1) * bs], ets[:], EXP)
                        # fill exp-paced tensor gaps with the previous
                        # block's value chains + pending conv matmuls
                        if vc0 is not None and nt % 2 == 1 and nt >= 3:
                            next(vc0, None)
                        pull_any(4)
                    # only one value generator may be live (single pz bank):
                    # finish this block's leftovers before queueing the next
                    if vc0 is not None:
                        for _ in vc0:
                            pass
                    while val_q:
                        pull_any(1)
                    prev_vals = value_chain(exp3, bs, m0, 1 if bi == 0 else 0)
                    for fn, k in ready.get(bi, []):
                        conv_queue.append(fn(k))
                # drain the last block's values and remaining conv work
                val_q.append(prev_vals)
                while val_q:
                    pull_any(1)
                pull_any(10 ** 9)

    nc.compile()
    return nc

# --------------------------------------------------------------------------
# entry point
# --------------------------------------------------------------------------

def _get_program():
    if 'nc' not in _PROGRAM_CACHE:
        _PROGRAM_CACHE['nc'] = build_program()
    return _PROGRAM_CACHE['nc']


def kernel(_trace=False, **inputs):
    nc = _get_program()
    in_maps = [host_prep(inputs, core) for core in range(8)]
    res = run_bass_kernel_spmd(nc, in_maps, core_ids=list(range(8)),
                               trace=_trace)
    y = np.zeros((B, C, H, W), np.float32)
    for core in range(8):
        b, s = core // 2, core % 2
        yh = res.results[core]['y']        # [2, 128, 32, 64]
        y[b, :, s * HALF:(s + 1) * HALF, :] = yh.reshape(C, HALF, 64)
    if _trace:
        return y, res
    return y


# revision 43
# speedup vs baseline: 1.0185x; 1.0034x over previous
"""Trainium2 Bass kernel for nn_Auto_Attn (self-attention + context flow + convs).

Sharding: 8 cores = (batch b in 0..3) x (image half s in 0..1). Each core
computes its half's 32 output rows plus 2 halo rows of the attention output
(conv1 3x3 -> conv2 3x3 needs a 2-row inp halo).

Per-core algorithm (validated against a float64 reference in numpy):
  q = wq @ x (1x1 conv);  energy[m,n] = q_m . q_n  (symmetric since key==query)
  Softmax rows are computed TRANSPOSED (eT[n,m], n on partitions) so the
  attention einsums need no transposes of the big attn matrix:
    - shift-invariance: softmax(e[m,:]) == softmax(e[m,:] - d[m]) for
      d[m] = e[m,m]; Cauchy-Schwarz bounds e[m,n]-d[m] <= (e[n,n]-e[m,m])/2
      so exp never overflows. The shift is folded into the energy matmul
      via an augmented contraction row: q_aug_n = [q; 1], q_aug_m = [q; -d].
    - Z[m] from a ones-column matmul against expT (pz).
  out/cf = expT.T @ [v | pre] in fp8 (e4m3) with DoubleRow perf mode
  (2 contraction rows/cycle); epilogue applies 1/Z, gamma/alpha/mask vectors,
  writes the conv input buffer via PE transposes into a zero-padded
  [38 x 66] bf16 spatial layout.

  Convs are 9-tap shifted bf16 matmuls over the padded layout, emitted in
  row-aligned chunks INTERLEAVED into the attention block loop: conv matmuls
  fill the tensor-engine gaps during the scalar-bound energy->exp phases,
  keeping the PE at full p-state. Outputs stream to DRAM per conv2 chunk.

dtypes: fp32r energy; fp8 e4m3 value matmuls (errors damped by gamma/alpha
~0.1); bf16 everywhere else. Validated offline: rel err ~3e-3 vs 2e-2 gate.
"""
import numpy as np
from contextlib import ExitStack

import ml_dtypes
import concourse.bass as bass
import concourse.tile as tile
from concourse import bacc, mybir
from concourse.bass_utils import run_bass_kernel_spmd
from concourse.masks import make_identity

F32 = mybir.dt.float32
F32R = mybir.dt.float32r
BF16 = mybir.dt.bfloat16
FP8 = mybir.dt.float8e4
DR = mybir.MatmulPerfMode.DoubleRow
EXP = mybir.ActivationFunctionType.Exp
PRELU = mybir.ActivationFunctionType.Prelu
IDENT = mybir.ActivationFunctionType.Identity

B, C, H, W = 4, 256, 64, 64
N = H * W                # 4096
CQ = 64
HALF = 32
WIN = 36                 # window rows (32 + 2 halo each side)
BUFR, BUFC = 38, 66      # padded conv buffer: +1 guard row / +1 pad col each side
MW = WIN * 64            # 2304 m positions per core
NT = N // 128            # 32 n-tiles
MCH = MW // 128          # 18 m-chunks
R1 = [(2, 6), (8, 6), (14, 6), (20, 6), (26, 6), (32, 4)]   # conv1 row chunks
R2 = [(3, 6), (9, 6), (15, 6), (21, 6), (27, 6), (33, 2)]   # conv2 row chunks

_PROGRAM_CACHE = {}


def tap_off(t):
    return (t // 3 - 1) * BUFC + (t % 3 - 1)


# --------------------------------------------------------------------------
# host-side prep
# --------------------------------------------------------------------------

def host_prep(inputs, core):
    b, s = core // 2, core % 2
    f32 = np.float32
    bf16 = ml_dtypes.bfloat16
    fp8 = ml_dtypes.float8_e4m3
    x = np.asarray(inputs['x'][b], f32).reshape(C, N)
    pre = np.asarray(inputs['pre'][b], f32).reshape(C, N)
    M = np.asarray(inputs['mask'][b, 0], f32).reshape(N)
    gamma = float(np.asarray(inputs['gamma']).reshape(-1)[0])
    alpha = float(np.asarray(inputs['alpha']).reshape(-1)[0])

    r_lo = s * HALF - 2
    rows = np.arange(r_lo, r_lo + WIN)
    valid_rows = (rows >= 0) & (rows < H)
    valid = np.repeat(valid_rows, 64).astype(f32)

    def win_slice(t):
        o = np.zeros((t.shape[0], MW), f32)
        vr = np.where(valid_rows)[0]
        o[:, vr[0] * 64:vr[-1] * 64 + 64] = t[:, rows[vr[0]] * 64: rows[vr[-1]] * 64 + 64]
        return o

    xm = win_slice(x)
    pm = win_slice(pre)
    Mw = win_slice(M[None, :])[0]

    vpT = np.zeros((N, 2 * C), f32)
    vpT[:, 0:C] = x.T
    vpT[:, C:2 * C] = pre.T

    xTm = np.ascontiguousarray(xm.T)    # [MW, C]
    pTm = np.ascontiguousarray(pm.T)

    def pervec(v):  # [MW] -> [128, 18] (per-partition layout per m-chunk)
        return np.ascontiguousarray(v.reshape(MCH, 128).T)

    wq = np.asarray(inputs['wq'], f32).reshape(CQ, C)
    w1 = np.asarray(inputs['w1'], f32)
    w2 = np.asarray(inputs['w2'], f32)
    ws = np.asarray(inputs['ws'], f32).reshape(C, 2 * C)
    bq = np.asarray(inputs['bq'], f32).reshape(CQ, 1)
    b1 = np.asarray(inputs['b1'], f32)
    b2 = np.asarray(inputs['b2'], f32)
    bs = np.asarray(inputs['bs'], f32)

    br_rows = r_lo + np.arange(BUFR) - 1
    rv = ((br_rows >= 0) & (br_rows < H)).astype(f32)
    cv = np.zeros(BUFC, f32)
    cv[1:65] = 1.0
    hrmask = np.broadcast_to((rv[:, None] * cv[None, :]).reshape(1, -1),
                             (128, BUFR * BUFC))

    return {
        'xb': x.reshape(2, 128, N).astype(bf16),
        'xm': xm.reshape(2, 128, MW).astype(bf16),
        'vpT': vpT.astype(fp8).reshape(NT, 128, 2 * C),
        'vones': np.ones((128, NT), fp8),
        'xTm': np.ascontiguousarray(xTm.reshape(MCH, 128, C).transpose(1, 0, 2)
                                    .reshape(128, MCH * C)).astype(bf16),
        'pTm': np.ascontiguousarray(pTm.reshape(MCH, 128, C).transpose(1, 0, 2)
                                    .reshape(128, MCH * C)).astype(bf16),
        'avec': pervec(gamma * valid),
        'b1vec': pervec(alpha * Mw * valid),
        'b2vec': pervec((1.0 - Mw) * valid),
        'wq': np.ascontiguousarray(wq.T).reshape(2, 128, CQ).astype(bf16),
        'bq': bq,
        'w1': np.ascontiguousarray(np.transpose(w1, (2, 3, 1, 0))).reshape(9, 4, 128, C).astype(bf16),
        'w2': np.ascontiguousarray(np.transpose(w2, (2, 3, 1, 0))).reshape(9, 2, 128, C).astype(bf16),
        'ws': np.ascontiguousarray(ws.T).reshape(4, 128, C).astype(bf16),
        'b1c': np.ascontiguousarray(b1.reshape(2, 128).T),
        'bfin': np.ascontiguousarray((b2 + bs).reshape(2, 128).T),
        'hrmask': np.ascontiguousarray(hrmask).astype(bf16),
        'ones_row': np.ones((1, N), bf16),
        'ones_col': np.ones((CQ, 1), bf16),
    }


# --------------------------------------------------------------------------
# device program
# --------------------------------------------------------------------------

def build_program():
    nc = bacc.Bacc("TRN2", target_bir_lowering=False, debug=False, num_devices=8)

    def din(name, shape, dt=F32):
        return nc.dram_tensor(name, shape, dt, kind="ExternalInput").ap()

    xb_d = din('xb', [2, 128, N], BF16)
    xm_d = din('xm', [2, 128, MW], BF16)
    vpT_d = din('vpT', [NT, 128, 2 * C], FP8)
    vones_d = din('vones', [128, NT], FP8)
    xTm_d = din('xTm', [128, MCH * C], BF16)
    pTm_d = din('pTm', [128, MCH * C], BF16)
    avec_d = din('avec', [128, MCH])
    b1vec_d = din('b1vec', [128, MCH])
    b2vec_d = din('b2vec', [128, MCH])
    wq_d = din('wq', [2, 128, CQ], BF16)
    bq_d = din('bq', [CQ, 1])
    w1_d = din('w1', [9, 4, 128, C], BF16)
    w2_d = din('w2', [9, 2, 128, C], BF16)
    ws_d = din('ws', [4, 128, C], BF16)
    b1c_d = din('b1c', [128, 2])
    bfin_d = din('bfin', [128, 2])
    hrmask_d = din('hrmask', [128, BUFR * BUFC], BF16)
    onesr_d = din('ones_row', [1, N], BF16)
    onesc_d = din('ones_col', [CQ, 1], BF16)
    y_d = nc.dram_tensor('y', [2, 128, HALF, 64], F32, kind="ExternalOutput").ap()

    with tile.TileContext(nc) as tc, ExitStack() as ctx:
        # ---------------- persistent tiles ----------------
        persist = ctx.enter_context(tc.tile_pool(name="persist", bufs=1))
        inp_raw = [persist.tile([128, BUFR * BUFC], BF16, tag=f"inp{i}", name=f"inp{i}")
                   for i in range(4)]
        h1 = [persist.tile([128, BUFR * BUFC], BF16, tag=f"h1{i}", name=f"h1{i}")
              for i in range(2)]
        w1_t = persist.tile([128, 36 * C], BF16, tag="w1")
        w2_t = persist.tile([128, 18 * C], BF16, tag="w2")
        ws_t = persist.tile([128, 4 * C], BF16, tag="ws")
        b1c_t = persist.tile([128, 2], F32, tag="b1c")
        bfin_t = persist.tile([128, 2], F32, tag="bfin")
        hrm_t = persist.tile([128, BUFR * BUFC], BF16, tag="hrm")
        y_sb = [persist.tile([128, HALF * 64], F32, tag=f"ysb{i}", name=f"ysb{i}")
                for i in range(2)]

        with tc.tile_pool(name="attn", bufs=1) as ap:
            q_aug_n = ap.tile([CQ + 1, N], BF16, tag="qn")
            q_aug_m = ap.tile([CQ + 1, MW], BF16, tag="qm")
            vpT_t = ap.tile([128, NT * 2 * C], FP8, tag="vpT")
            vones_t = ap.tile([128, NT], FP8, tag="vones")
            xTmA = ap.tile([128, MCH * C], BF16, tag="xTmA")
            pTmA = ap.tile([128, MCH * C], BF16, tag="pTmA")
            avec_t = ap.tile([128, MCH], F32, tag="av")
            b1vec_t = ap.tile([128, MCH], F32, tag="b1v")
            b2vec_t = ap.tile([128, MCH], F32, tag="b2v")
            wq_t = ap.tile([128, 2 * CQ], BF16, tag="wq")
            xb_t = ap.tile([128, 2 * N], BF16, tag="xbt")
            xm_t = ap.tile([128, 2 * MW], BF16, tag="xmt")
            bq_t = ap.tile([CQ, 1], F32, tag="bq")
            onesq = ap.tile([CQ, 1], BF16, tag="onesq")
            ident = ap.tile([128, 128], BF16, tag="id")

            nc.sync.dma_start(bq_t[:], bq_d)
            nc.sync.dma_start(onesq[:], onesc_d)
            for ck in range(2):
                nc.sync.dma_start(wq_t[:, ck * CQ:(ck + 1) * CQ], wq_d[ck])
            nc.sync.dma_start(q_aug_n[CQ:CQ + 1, :], onesr_d)
            make_identity(nc, ident[:])

            # ---------------- q phase ----------------
            with tc.tile_pool(name="qp1", bufs=1) as qp1, \
                 tc.tile_pool(name="qps", bufs=2, space="PSUM") as qps:
                # coarse input streams: 256KB+ transfers, interleaved so the
                # first q matmuls unblock as early as possible
                QCH = 1024
                for i in range(N // QCH):
                    for ck in range(2):
                        nc.sync.dma_start(
                            xb_t[:, ck * N + i * QCH:ck * N + (i + 1) * QCH],
                            xb_d[ck][:, i * QCH:(i + 1) * QCH])
                for i in range(2):
                    for ck in range(2):
                        nc.sync.dma_start(
                            xm_t[:, ck * MW + i * (MW // 2):ck * MW + (i + 1) * (MW // 2)],
                            xm_d[ck][:, i * (MW // 2):(i + 1) * (MW // 2)])
                for nb in range(8):
                    qsum = qps.tile([CQ, 512], F32, tag="qpsum")
                    for ck in range(2):
                        nc.tensor.matmul(qsum[:], wq_t[:, ck * CQ:(ck + 1) * CQ],
                                         xb_t[:, ck * N + nb * 512:ck * N + (nb + 1) * 512],
                                         start=(ck == 0), stop=(ck == 1))
                    nc.vector.tensor_scalar_add(
                        q_aug_n[0:CQ, nb * 512:(nb + 1) * 512], qsum[:], bq_t[:])

                mchunks = [512, 512, 512, 512, 256]
                qsq = qp1.tile([CQ, MW], BF16, tag="qsq")
                pend = None

                def emit_d(mo, cs):
                    dps = qps.tile([1, 512], F32, tag="dpsum")
                    nc.tensor.matmul(dps[:, 0:cs], onesq[:], qsq[:, mo:mo + cs],
                                     start=True, stop=True)
                    nc.vector.tensor_scalar_mul(
                        q_aug_m[CQ:CQ + 1, mo:mo + cs], dps[:, 0:cs], -1.0)

                mo = 0
                for cs in mchunks:
                    qsum = qps.tile([CQ, 512], F32, tag="qpsum")
                    for ck in range(2):
                        nc.tensor.matmul(qsum[:, 0:cs], wq_t[:, ck * CQ:(ck + 1) * CQ],
                                         xm_t[:, ck * MW + mo:ck * MW + mo + cs],
                                         start=(ck == 0), stop=(ck == 1))
                    if pend is not None:
                        emit_d(*pend)   # previous chunk's d: hides vector latency
                    nc.vector.tensor_scalar_add(
                        q_aug_m[0:CQ, mo:mo + cs], qsum[:, 0:cs], bq_t[:])
                    nc.vector.tensor_mul(qsq[:, mo:mo + cs], q_aug_m[0:CQ, mo:mo + cs],
                                         q_aug_m[0:CQ, mo:mo + cs])
                    pend = (mo, cs)
                    mo += cs
                emit_d(*pend)

                # bulk loads issued now: they queue behind the q streams and
                # overlap the attention phase
                nc.sync.dma_start(vones_t[:], vones_d)
                for nt in range(NT):
                    nc.sync.dma_start(vpT_t[:, nt * 2 * C:(nt + 1) * 2 * C], vpT_d[nt])
                nc.sync.dma_start(xTmA[:], xTm_d)
                nc.sync.dma_start(pTmA[:], pTm_d)
                nc.sync.dma_start(avec_t[:], avec_d)
                nc.sync.dma_start(b1vec_t[:], b1vec_d)
                nc.sync.dma_start(b2vec_t[:], b2vec_d)
                for t in range(9):
                    for ick in range(4):
                        nc.sync.dma_start(w1_t[:, (t * 4 + ick) * C:(t * 4 + ick + 1) * C],
                                          w1_d[t, ick])
                for t in range(9):
                    for ick in range(2):
                        nc.sync.dma_start(w2_t[:, (t * 2 + ick) * C:(t * 2 + ick + 1) * C],
                                          w2_d[t, ick])
                for ick in range(4):
                    nc.sync.dma_start(ws_t[:, ick * C:(ick + 1) * C], ws_d[ick])
                nc.sync.dma_start(hrm_t[:], hrmask_d)
                nc.sync.dma_start(b1c_t[:], b1c_d)
                nc.sync.dma_start(bfin_t[:], bfin_d)

            # padded-buffer zeroing: off the startup critical path, done
            # well before the first epilogue/conv reads
            for t_ in inp_raw + h1:
                nc.vector.memset(t_[:], 0.0)

            # ---------------- attention blocks + interleaved convs ---------
            with tc.tile_pool(name="expp", bufs=2) as expp, \
                 tc.tile_pool(name="ep", bufs=2) as epp, \
                 tc.tile_pool(name="lrp", bufs=2) as lrp, \
                 tc.tile_pool(name="eps", bufs=2, space="PSUM") as eps, \
                 tc.tile_pool(name="ops", bufs=2, space="PSUM") as ops, \
                 tc.tile_pool(name="zps", bufs=1, space="PSUM") as zps, \
                 tc.tile_pool(name="tps", bufs=1, space="PSUM") as tps, \
                 tc.tile_pool(name="c1ps", bufs=1, space="PSUM") as c1ps, \
                 tc.tile_pool(name="c2ps", bufs=1, space="PSUM") as c2ps:

                def conv1_chunk(k):
                    r0, nr = R1[k]
                    base, cs = r0 * BUFC, nr * BUFC
                    lr = [lrp.tile([128, 536], BF16, tag=f"lr{i}", name=f"lr{i}")
                          for i in range(4)]
                    lrt = lrp.tile([128, 536], BF16, tag="lrt", name="lrt")
                    for ick in range(2):
                        yield nc.scalar.activation(lr[ick][:, 0:cs + 134],
                                                   inp_raw[ick][:, base - 67:base + cs + 67],
                                                   PRELU, alpha=0.1)
                    for ick in range(2, 4):
                        # prelu on vector: max(x,0) + 0.1*min(x,0)
                        src = inp_raw[ick][:, base - 67:base + cs + 67]
                        yield nc.vector.tensor_scalar_max(lr[ick][:, 0:cs + 134], src, 0.0)
                        yield nc.vector.tensor_scalar_min(lrt[:, 0:cs + 134], src, 0.0)
                        yield nc.vector.scalar_tensor_tensor(
                            lr[ick][:, 0:cs + 134], lrt[:, 0:cs + 134], 0.1,
                            lr[ick][:, 0:cs + 134],
                            op0=mybir.AluOpType.mult, op1=mybir.AluOpType.add)
                    for oc in range(2):
                        ps1 = c1ps.tile([128, 396], F32, tag="ps1")
                        k_ = 0
                        for t in range(9):
                            off = 67 + tap_off(t)
                            for ick in range(4):
                                yield nc.tensor.matmul(
                                    ps1[:, 0:cs],
                                    w1_t[:, (t * 4 + ick) * C + oc * 128:
                                         (t * 4 + ick) * C + oc * 128 + 128],
                                    lr[ick][:, off:off + cs],
                                    start=(k_ == 0), stop=(k_ == 35))
                                k_ += 1
                        yield nc.scalar.activation(h1[oc][:, base:base + cs], ps1[:, 0:cs],
                                                   PRELU, bias=b1c_t[:, oc:oc + 1],
                                                   alpha=0.1)
                        yield nc.vector.tensor_mul(h1[oc][:, base:base + cs],
                                                   h1[oc][:, base:base + cs],
                                                   hrm_t[:, base:base + cs])

                def conv2_chunk(k):
                    r0, nr = R2[k]
                    base, cs = r0 * BUFC, nr * BUFC
                    for oc in range(2):
                        ps2 = c2ps.tile([128, 396], F32, tag="ps2")
                        k_ = 0
                        for ick in range(4):
                            yield nc.tensor.matmul(
                                ps2[:, 0:cs],
                                ws_t[:, ick * C + oc * 128:ick * C + oc * 128 + 128],
                                inp_raw[ick][:, base:base + cs],
                                start=(k_ == 0), stop=False)
                            k_ += 1
                        for t in range(9):
                            off = tap_off(t)
                            for ick in range(2):
                                k_ += 1
                                yield nc.tensor.matmul(
                                    ps2[:, 0:cs],
                                    w2_t[:, (t * 2 + ick) * C + oc * 128:
                                         (t * 2 + ick) * C + oc * 128 + 128],
                                    h1[ick][:, base + off:base + off + cs],
                                    start=False, stop=(k_ == 22))
                        src = ps2[:, 0:cs].rearrange("p (r c) -> p r c", c=BUFC)[:, :, 1:65]
                        dst = y_sb[oc][:, (r0 - 3) * 64:(r0 - 3 + nr) * 64] \
                            .rearrange("p (r c) -> p r c", c=64)
                        yield nc.vector.tensor_scalar_add(dst, src,
                                                          bfin_t[:, oc:oc + 1])
                        yield nc.sync.dma_start(
                            y_d[oc][:, r0 - 3:r0 - 3 + nr, :], dst)

                conv_queue = []

                def pull_conv(n):
                    done = 0
                    while conv_queue and done < n:
                        try:
                            next(conv_queue[0])
                            done += 1
                        except StopIteration:
                            conv_queue.pop(0)

                ready = {0: [(conv1_chunk, 0)],
                         1: [(conv1_chunk, 1), (conv2_chunk, 0)],
                         2: [(conv1_chunk, 2), (conv2_chunk, 1)],
                         3: [(conv1_chunk, 3), (conv2_chunk, 2),
                             (conv1_chunk, 4), (conv2_chunk, 3)],
                         4: [(conv1_chunk, 5), (conv2_chunk, 4), (conv2_chunk, 5)]}

                blocks = [(0, 512), (512, 512), (1024, 512), (1536, 512), (2048, 256)]
                vp3 = vpT_t[:].rearrange("p (t v) -> p t v", v=2 * C)
                def value_chain(exp3, bs, m0, mc_lo=0, mc_hi=None):
                    for mc in range(mc_lo, mc_hi if mc_hi is not None else bs // 128):
                        j = m0 // 128 + mc
                        po = ops.tile([128, 512], F32, tag="po")
                        pz = zps.tile([128, 1], F32, tag="pz")
                        for t in range(NT // 2):
                            lhs = exp3[:, 2 * t:2 * t + 2, mc * 128:mc * 128 + 128]
                            nc.tensor.matmul(
                                po[:], lhs, vp3[:, 2 * t:2 * t + 2, :],
                                start=(t == 0), stop=(t == NT // 2 - 1),
                                perf_mode=DR)
                            for u in (2 * t, 2 * t + 1):
                                nc.tensor.matmul(
                                    pz[:], exp3[:, u, mc * 128:mc * 128 + 128],
                                    vones_t[:, u:u + 1],
                                    start=(u == 0), stop=(u == NT - 1))
                            yield
                        # epilogue
                        rt = epp.tile([128, 1], F32, tag="rt")
                        nc.vector.reciprocal(rt[:], pz[:, 0:1])
                        s1 = epp.tile([128, 1], F32, tag="s1")
                        s2 = epp.tile([128, 1], F32, tag="s2")
                        nc.vector.tensor_mul(s1[:], rt[:], avec_t[:, j:j + 1])
                        nc.vector.tensor_mul(s2[:], rt[:], b1vec_t[:, j:j + 1])
                        yield
                        ov = epp.tile([128, C], BF16, tag="ov")
                        cv = epp.tile([128, C], BF16, tag="cv")
                        tv = epp.tile([128, C], BF16, tag="tv")
                        nc.vector.tensor_scalar_mul(ov[:], po[:, 0:C], s1[:])
                        nc.vector.tensor_add(ov[:], ov[:], xTmA[:, j * C:(j + 1) * C])
                        yield
                        nc.vector.tensor_scalar_mul(cv[:], po[:, C:2 * C], s2[:])
                        nc.vector.tensor_scalar_mul(tv[:], pTmA[:, j * C:(j + 1) * C],
                                                    b2vec_t[:, j:j + 1])
                        nc.vector.tensor_add(cv[:], cv[:], tv[:])
                        yield
                        # transpose [m, c] -> [c, m] into padded conv layout
                        br0 = 1 + 2 * j
                        for h_ in range(4):
                            src_ = (ov if h_ < 2 else cv)
                            pt = tps.tile([128, 128], BF16, tag="pt")
                            nc.tensor.transpose(pt[:], src_[:, (h_ % 2) * 128:(h_ % 2) * 128 + 128],
                                                ident[:])
                            dst = inp_raw[h_][:].rearrange("p (r c) -> p r c", c=BUFC)
                            nc.vector.tensor_copy(dst[:, br0:br0 + 2, 1:65],
                                                  pt[:].rearrange("p (r c) -> p r c", c=64))
                            yield

                # unified work queue: value/epilogue generators go to the
                # front (they unblock conv-input rows), conv chunks append
                def pull_work(n):
                    done = 0
                    while conv_queue and done < n:
                        try:
                            next(conv_queue[0])
                            done += 1
                        except StopIteration:
                            conv_queue.pop(0)

                val_q = []

                def pull_any(n):
                    done = 0
                    while done < n and (val_q or conv_queue):
                        q_ = val_q if val_q else conv_queue
                        try:
                            next(q_[0])
                            done += 1
                        except StopIteration:
                            q_.pop(0)

                prev_vals = None
                for bi, (m0, bs) in enumerate(blocks):
                    expT = expp.tile([128, NT * 512], FP8, tag="expT", name="expT")
                    exp3 = expT[:].rearrange("p (t b) -> p t b", b=bs)
                    if prev_vals is not None:
                        val_q.append(prev_vals)
                    vc0 = value_chain(exp3, bs, m0, 0, 1) if bi == 0 else None
                    for nt in range(NT):
                        ets = eps.tile([128, bs], F32, tag="et", name="ets")
                        nc.tensor.matmul(ets[:], q_aug_n[:, nt * 128:(nt + 1) * 128],
                                         q_aug_m[:, m0:m0 + bs],
                                         start=True, stop=True)
                        nc.scalar.activation(expT[:, nt * bs:(nt + 1) * bs], ets[:], EXP)
                        # fill exp-paced tensor gaps with the previous
                        # block's value chains + pending conv matmuls
                        if vc0 is not None and nt % 2 == 1 and nt >= 3:
                            next(vc0, None)
                        pull_any(4)
                    # only one value generator may be live (single pz bank):
                    # finish this block's leftovers before queueing the next
                    if vc0 is not None:
                        for _ in vc0:
                            pass
                    while val_q:
                        pull_any(1)
                    prev_vals = value_chain(exp3, bs, m0, 1 if bi == 0 else 0)
                    for fn, k in ready.get(bi, []):
                        conv_queue.append(fn(k))
                # drain the last block's values and remaining conv work
                val_q.append(prev_vals)
                while val_q:
                    pull_any(1)
                pull_any(10 ** 9)

    nc.compile()
    return nc

# --------------------------------------------------------------------------
# entry point
# --------------------------------------------------------------------------

def _get_program():
    if 'nc' not in _PROGRAM_CACHE:
        _PROGRAM_CACHE['nc'] = build_program()
    return _PROGRAM_CACHE['nc']


def kernel(_trace=False, **inputs):
    nc = _get_program()
    in_maps = [host_prep(inputs, core) for core in range(8)]
    res = run_bass_kernel_spmd(nc, in_maps, core_ids=list(range(8)),
                               trace=_trace)
    y = np.zeros((B, C, H, W), np.float32)
    for core in range(8):
        b, s = core // 2, core % 2
        yh = res.results[core]['y']        # [2, 128, 32, 64]
        y[b, :, s * HALF:(s + 1) * HALF, :] = yh.reshape(C, HALF, 64)
    if _trace:
        return y, res
    return y
